# revision 18
# baseline (speedup 1.0000x reference)
"""DiscConv (gnn_message_passing, sequential +/-1 edges) on 8 TRN2 cores.

The edge list produced by the oracle is the sequential +/-1 neighbor graph:
    src = [0..N-2, 1..N-1], dst = [1..N-1, 0..N-2]
so   widx = mod(src-dst, 3) = 2 for (j -> j+1) edges, 1 for (j+1 -> j) edges
and the whole op collapses to a depthwise 3-tap stencil along the node axis:
    out[i] = w0*x[i] + w2*x[i-1] + w1*x[i+1]      (elementwise per feature)

Strategy: graph-partition 125k nodes/core across 8 cores, halo = 1 node on
each side (zero-padded at the global boundary).  On host each shard is packed
FEATURE-ON-PARTITIONS: [128, 62502] where partition p = (half h = p//64,
feature f = p%64) and the free axis is the node index inside the half.  In
that layout the per-feature weights are per-partition scalars, so the stencil
is 3 vector-engine ops per tile (tensor_scalar_mul at the 2x_2P perf mode +
2 fused scalar_tensor_tensor mult-adds) with node shifts expressed as
free-dim offsets into the same SBUF tile.  All DMAs are fully contiguous
~1.3MB transfers; per core the kernel moves 32MB in + 32MB out, and the
cost-model timeline puts it at ~182us/core vs a ~178us pure-DMA bound.
"""

import numpy as np

N = 1_000_000
F = 64
M = 8                  # cores
NPC = N // M           # nodes per core = 125000
NH = NPC // 2          # nodes per partition-half = 62500
CT = 2_500             # tile width (free-dim columns per compute tile)
                       # must be EVEN: DVE 2x_2P perf mode needs even dims

TRACE = False          # set True (e.g. from test.py) to capture an NTFF trace
LAST_RESULT = None     # BassKernelResults of the most recent device run

_NC_CACHE = {}


def _build_bass(ct=CT, xbufs=4, obufs=4, repeat=1, mode="dve"):
    """Build the Bass/Tile program once per process.

    mode="dve" (default): all three ops on DVE (tensor_scalar_mul at 2x_2P
        + 2 fused STT).  DVE busy ~167us/core; cost model 182.0us/core —
        equal to the pure-DMA pipeline floor for 64MB/core of traffic.
    mode="act": insurance variant if the DVE 2x_2P perf mode ever fails to
        engage on silicon — ACT computes m1 = w1*x[i+1] (scale-copy), DVE
        does two fused STT mult-adds (plain 1x ops, no perf-mode
        assumptions), stores ride SWDGE.  DVE busy ~133us/core; cost model
        185.4us/core (cross-engine sem hops).  HW-validated (8.4e-8).
    """
    import concourse.tile as tile
    from concourse import bacc, mybir

    nc = bacc.Bacc("TRN2", debug=False, num_devices=M)
    x_in = nc.dram_tensor("xsh", [128, NH + 2], mybir.dt.float32,
                          kind="ExternalInput").ap()
    wv_in = nc.dram_tensor("wv", [128, 4], mybir.dt.float32,
                           kind="ExternalInput").ap()
    out_d = nc.dram_tensor("out", [128, NH], mybir.dt.float32,
                           kind="ExternalOutput").ap()

    mult = mybir.AluOpType.mult
    add = mybir.AluOpType.add

    if isinstance(ct, int):
        assert NH % ct == 0
        widths = [ct] * (NH // ct)
    else:
        widths = list(ct)
        assert sum(widths) == NH
    ctmax = max(widths)
    with tile.TileContext(nc) as tc:
        with tc.tile_pool(name="wpool", bufs=1) as wpool, \
             tc.tile_pool(name="xpool", bufs=xbufs) as xpool, \
             tc.tile_pool(name="apool", bufs=2) as apool, \
             tc.tile_pool(name="opool", bufs=obufs) as opool:
            # Load weights, then sink the DMA wait into a DVE copy so no
            # compute instruction ever needs a second semaphore wait slot
            # (TensorScalarPtr codegen allows only one sync-wait).
            wvs = wpool.tile([128, 4], mybir.dt.float32)
            nc.sync.dma_start(wvs[:], wv_in[:])
            wv = wpool.tile([128, 4], mybir.dt.float32)
            nc.vector.tensor_copy(wv[:], wvs[:])
            w0 = wv[:, 0:1]
            w1 = wv[:, 1:2]
            w2 = wv[:, 2:3]
            col = 0
            for w_t in widths * repeat:
                if col == NH:
                    col = 0
                xt = xpool.tile([128, ctmax + 2], mybir.dt.float32,
                                tag="xt")
                nc.sync.dma_start(xt[:, :w_t + 2], x_in[:, col: col + w_t + 2])
                # acc is only ever touched by DVE (no DMA WAR waits on it);
                # the final fused op writes ot, the only tile the store DMA
                # reads, so the store-WAR wait lands on that op alone.
                acc = apool.tile([128, ctmax], mybir.dt.float32, tag="acc")
                ot = opool.tile([128, ctmax], mybir.dt.float32, tag="ot")
                if mode == "act":
                    # acc = w1 * x[i+1]   (scalar engine: copy with scale)
                    nc.scalar.mul(acc[:, :w_t], xt[:, 2:w_t + 2], w1)
                    # acc = w0 * x[i] + acc
                    nc.vector.scalar_tensor_tensor(
                        acc[:, :w_t], xt[:, 1:w_t + 1], w0, acc[:, :w_t],
                        mult, add)
                    # ot = w2 * x[i-1] + acc
                    nc.vector.scalar_tensor_tensor(
                        ot[:, :w_t], xt[:, 0:w_t], w2, acc[:, :w_t],
                        mult, add)
                else:
                    # acc = w0 * x[i]
                    nc.vector.tensor_scalar_mul(acc[:, :w_t],
                                                xt[:, 1:w_t + 1], w0)
                    # acc += w2 * x[i-1]
                    nc.vector.scalar_tensor_tensor(
                        acc[:, :w_t], xt[:, 0:w_t], w2, acc[:, :w_t],
                        mult, add)
                    # ot = w1 * x[i+1] + acc
                    nc.vector.scalar_tensor_tensor(
                        ot[:, :w_t], xt[:, 2:w_t + 2], w1, acc[:, :w_t],
                        mult, add)
                # Stores ride a ring whose engine does no compute, so their
                # waits on DVE never head-of-line-block compute dispatch:
                # ACT ring in "dve" mode, SWDGE (Pool) ring in "act" mode.
                st_eng = nc.gpsimd if mode == "act" else nc.scalar
                st_eng.dma_start(out_d[:, col: col + w_t], ot[:, :w_t])
                col += w_t
    nc.compile()
    return nc


def _edges_are_sequential(disc_edges) -> bool:
    if disc_edges.shape != (2, 2 * (N - 1)):
        return False
    idx = np.arange(N, dtype=disc_edges.dtype)
    src, dst = disc_edges[0], disc_edges[1]
    return (np.array_equal(src[:N - 1], idx[:-1])
            and np.array_equal(src[N - 1:], idx[1:])
            and np.array_equal(dst[:N - 1], idx[1:])
            and np.array_equal(dst[N - 1:], idx[:-1]))


def _fallback(x, disc_edges, weight):
    """General-edge reference path (host, numpy) — only used if the edge
    list ever deviates from the sequential +/-1 pattern."""
    src = disc_edges[0].astype(np.int64)
    dst = disc_edges[1].astype(np.int64)
    widx = np.mod(src - dst, weight.shape[0])
    msg = weight[widx] * x[src]
    order = np.argsort(dst, kind="stable")
    ds = dst[order]
    msgs = msg[order]
    out = weight[0] * x
    if ds.size:
        bounds = np.flatnonzero(np.diff(ds)) + 1
        seg_starts = np.concatenate(([0], bounds))
        sums = np.add.reduceat(msgs, seg_starts, axis=0)
        out[ds[seg_starts]] += sums.astype(np.float32)
    return out.astype(np.float32)


def kernel(x, disc_edges, weight):
    global LAST_RESULT
    x = np.ascontiguousarray(np.asarray(x, dtype=np.float32))
    disc_edges = np.asarray(disc_edges)
    weight = np.asarray(weight, dtype=np.float32)

    if x.shape != (N, F) or not _edges_are_sequential(disc_edges):
        return _fallback(x, disc_edges, weight)

    from concourse.bass_utils import run_bass_kernel_spmd

    if "nc" not in _NC_CACHE:
        _NC_CACHE["nc"] = _build_bass()
    nc = _NC_CACHE["nc"]

    # --- host-side shard packing (feature-on-partitions, 1-node halos) ---
    xs = np.zeros((M, 128, NH + 2), np.float32)
    for c in range(M):
        for h in range(2):
            s = c * NPC + h * NH
            lo, hi = s - 1, s + NH + 1
            a, b = max(lo, 0), min(hi, N)
            xs[c, h * 64:(h + 1) * 64, a - lo:(a - lo) + (b - a)] = x[a:b, :].T

    wvh = np.zeros((128, 4), np.float32)
    for d in range(3):
        wvh[0:64, d] = weight[d]
        wvh[64:128, d] = weight[d]

    in_maps = [{"xsh": xs[c], "wv": wvh} for c in range(M)]
    try:
        res = run_bass_kernel_spmd(nc, in_maps, core_ids=list(range(M)),
                                   trace=TRACE)
    except (ImportError, ModuleNotFoundError):
        # NTFF trace hooks are absent in some containers; rerun untraced.
        res = run_bass_kernel_spmd(nc, in_maps, core_ids=list(range(M)),
                                   trace=False)
    LAST_RESULT = res

    out = np.empty((N, F), np.float32)
    for c in range(M):
        o = res.results[c]["out"]
        for h in range(2):
            s = c * NPC + h * NH
            out[s:s + NH, :] = o[h * 64:(h + 1) * 64, :].T
    return out
